# revision 12
# baseline (speedup 1.0000x reference)
"""AdaKQuantizer forward on 8 TRN2 NeuronCores — pure data parallel.

Per row of x[B=65536, Q=1024]:
  k   = argmax(x @ kdecider_w.T) + 1            (k in 1..64)
  t   = k-th largest value of the row
  mask= (x >= t)                                 (top-k mask)
  out = mask @ codebook_w.T                      ([B, 256])

Each core gets 8192 rows. Per 128-row tile:
  - PE-transpose x into [q, rows] chunks (needed: TensorE contracts over
    the partition axis for both matmuls)
  - kd matmul (fp32), argmax via max8 + is_ge one-hot
  - top-64 per row via 8 rounds of max8 + match_replace (DVE)
  - threshold t = <top64, onehot>; broadcast across partitions via a
    K=1 outer-product matmul; maskT = (xT >= t) in bf16
  - codebook matmul in bf16 (mask is exact 0/1), fp32 accumulate
"""

import sys

sys.path.insert(0, "/opt/trn_rl_repo")

import numpy as np

import concourse.bass as bass  # noqa: F401  (registers engines)
import concourse.mybir as mybir
import concourse.tile as tile
from concourse import bacc
from concourse.bass_utils import run_bass_kernel_spmd
from concourse.masks import make_identity

B, Q, E, MAXK = 65536, 1024, 256, 64
NCORES = 8
RPC = B // NCORES  # 8192 rows per core
TILE_ROWS = 128
NTILES = RPC // TILE_ROWS  # 64
NCHUNK = Q // 128  # 8
NEG = -1.0e30

_NC_CACHE = {}


def build_nc():
    nc = bacc.Bacc(None, target_bir_lowering=False)
    f32 = mybir.dt.float32
    bf16 = mybir.dt.bfloat16

    x_ext = nc.declare_dram_parameter("x", [RPC, Q], f32, isOutput=False)
    kdt_ext = nc.declare_dram_parameter("kdt", [Q, MAXK], f32, isOutput=False)
    cbt_ext = nc.declare_dram_parameter("cbt", [Q, E], f32, isOutput=False)
    out_ext = nc.declare_dram_parameter("out", [RPC, E], f32, isOutput=True)

    with tile.TileContext(nc) as tc:
        with (
            tc.tile_pool(name="consts", bufs=1) as cpool,
            tc.tile_pool(name="work", bufs=6) as wpool,
            tc.tile_pool(name="ps_xt", bufs=2, space="PSUM") as ppool_xt,
            tc.tile_pool(name="ps_mm", bufs=2, space="PSUM") as ppool_mm,
        ):
            ident = cpool.tile([128, 128], f32)
            make_identity(nc, ident)

            # kdecider_w.T [1024, 64] -> chunks on partitions: [128, 8*64]
            kdt_sb = cpool.tile([128, NCHUNK * MAXK], f32)
            nc.sync.dma_start(
                kdt_sb.rearrange("p (c n) -> p c n", c=NCHUNK),
                kdt_ext.rearrange("(c p) n -> p c n", p=128),
            )
            # codebook_w.T [1024, 256] -> [128, 8*256], converted to bf16
            cbt_f32 = cpool.tile([128, NCHUNK * E], f32)
            nc.sync.dma_start(
                cbt_f32.rearrange("p (c n) -> p c n", c=NCHUNK),
                cbt_ext.rearrange("(c p) n -> p c n", p=128),
            )
            cbt_sb = cpool.tile([128, NCHUNK * E], bf16)
            nc.vector.tensor_copy(cbt_sb, cbt_f32)

            for i in range(NTILES):
                r0 = i * TILE_ROWS
                x_sb = wpool.tile([128, Q], f32, tag="x")
                nc.sync.dma_start(x_sb, x_ext[r0 : r0 + TILE_ROWS, :])

                # ---- transpose x -> xT (8x PE transpose + copies out) ----
                xT_ps = ppool_xt.tile([128, Q], f32, tag="xT_ps")
                for c in range(NCHUNK):
                    nc.tensor.transpose(
                        xT_ps[:, c * 128 : (c + 1) * 128],
                        x_sb[:, c * 128 : (c + 1) * 128],
                        ident,
                    )
                xT_sb = wpool.tile([128, Q], f32, tag="xT")
                for c in range(NCHUNK):
                    nc.scalar.copy(
                        xT_sb[:, c * 128 : (c + 1) * 128],
                        xT_ps[:, c * 128 : (c + 1) * 128],
                    )

                # ---- kd = x @ kdT (fp32, accumulate over 8 chunks) ----
                kd_ps = ppool_mm.tile([128, MAXK], f32, tag="kd_ps")
                for c in range(NCHUNK):
                    nc.tensor.matmul(
                        kd_ps,
                        xT_sb[:, c * 128 : (c + 1) * 128],
                        kdt_sb[:, c * MAXK : (c + 1) * MAXK],
                        start=(c == 0),
                        stop=(c == NCHUNK - 1),
                    )
                kd_sb = wpool.tile([128, MAXK], f32, tag="kd_sb")
                nc.scalar.copy(kd_sb, kd_ps)

                # one-hot of argmax(kd) (row max; ties ~impossible)
                kdm8 = wpool.tile([128, 8], f32, tag="kdm8")
                nc.vector.max(out=kdm8, in_=kd_sb)
                oh = wpool.tile([128, MAXK], f32, tag="oh")
                nc.gpsimd.tensor_scalar(
                    oh, kd_sb, kdm8[:, 0:1], None, op0=mybir.AluOpType.is_ge
                )

                # ---- top-64 of each row: 8 rounds max8 + match_replace ----
                top64 = wpool.tile([128, MAXK], f32, tag="top64")
                wbuf = wpool.tile([128, Q], f32, tag="wbuf")
                cur = x_sb
                for g in range(8):
                    nc.vector.max(out=top64[:, g * 8 : (g + 1) * 8], in_=cur)
                    nc.vector.match_replace(
                        out=wbuf,
                        in_to_replace=top64[:, g * 8 : (g + 1) * 8],
                        in_values=cur,
                        imm_value=NEG,
                    )
                    cur = wbuf

                # ---- threshold t = sum(top64 * onehot)  [128, 1] ----
                prod = wpool.tile([128, MAXK], f32, tag="prod")
                nc.gpsimd.tensor_mul(prod, top64, oh)
                tthr = wpool.tile([128, 1], f32, tag="tthr")
                nc.vector.reduce_sum(tthr, prod, axis=mybir.AxisListType.X)

                # ---- mask in natural layout: mask[r, q] = x[r, q] >= t[r]
                # (per-partition scalar threshold, single DVE op, bf16 out)
                mask_nat = wpool.tile([128, Q], bf16, tag="mask_nat")
                nc.gpsimd.tensor_scalar(
                    mask_nat, x_sb, tthr[:, 0:1], None, op0=mybir.AluOpType.is_ge
                )

                # transpose mask via DMA xbar (2-byte dtype, HWDGE)
                maskT = wpool.tile([128, Q], bf16, tag="maskT")
                for c in range(NCHUNK):
                    eng = nc.sync if c % 2 == 0 else nc.scalar
                    eng.dma_start(
                        maskT[:, c * 128 : (c + 1) * 128],
                        mask_nat[:, c * 128 : (c + 1) * 128],
                        transpose=True,
                    )

                # ---- out = mask @ cbT (bf16 inputs, fp32 accumulate) ----
                out_ps = ppool_mm.tile([128, E], f32, tag="out_ps")
                for c in range(NCHUNK):
                    nc.tensor.matmul(
                        out_ps,
                        maskT[:, c * 128 : (c + 1) * 128],
                        cbt_sb[:, c * E : (c + 1) * E],
                        start=(c == 0),
                        stop=(c == NCHUNK - 1),
                    )
                out_sb = wpool.tile([128, E], f32, tag="out_sb")
                nc.scalar.copy(out_sb, out_ps)
                nc.sync.dma_start(out_ext[r0 : r0 + TILE_ROWS, :], out_sb)

    nc.finalize()
    return nc


def get_nc():
    if "nc" not in _NC_CACHE:
        _NC_CACHE["nc"] = build_nc()
    return _NC_CACHE["nc"]


def kernel(x, codebook_w, kdecider_w):
    x = np.ascontiguousarray(np.asarray(x, dtype=np.float32))
    kdt = np.ascontiguousarray(np.asarray(kdecider_w, dtype=np.float32).T)
    cbt = np.ascontiguousarray(np.asarray(codebook_w, dtype=np.float32).T)
    nc = get_nc()
    in_maps = [
        {"x": x[i * RPC : (i + 1) * RPC], "kdt": kdt, "cbt": cbt}
        for i in range(NCORES)
    ]
    res = run_bass_kernel_spmd(nc, in_maps, core_ids=list(range(NCORES)))
    return np.concatenate([res.results[i]["out"] for i in range(NCORES)], axis=0)


# revision 13
# speedup vs baseline: 1.4517x; 1.4517x over previous
"""AdaKQuantizer forward on 8 TRN2 NeuronCores — pure data parallel.

Per row of x[B=65536, Q=1024]:
  k   = argmax(x @ kdecider_w.T) + 1            (k in 1..64)
  t   = k-th largest value of the row
  mask= (x >= t)                                 (top-k mask)
  out = mask @ codebook_w.T                      ([B, 256])

Each core gets 8192 rows. Per 128-row tile:
  - PE-transpose x into [q, rows] chunks (needed: TensorE contracts over
    the partition axis for both matmuls)
  - kd matmul (fp32), argmax via max8 + is_ge one-hot
  - top-64 per row via 8 rounds of max8 + match_replace (DVE)
  - threshold t = <top64, onehot>; broadcast across partitions via a
    K=1 outer-product matmul; maskT = (xT >= t) in bf16
  - codebook matmul in bf16 (mask is exact 0/1), fp32 accumulate
"""

import sys

sys.path.insert(0, "/opt/trn_rl_repo")

import numpy as np

import concourse.bass as bass  # noqa: F401  (registers engines)
import concourse.mybir as mybir
import concourse.tile as tile
from concourse import bacc
from concourse.bass_utils import run_bass_kernel_spmd
from concourse.masks import make_identity

B, Q, E, MAXK = 65536, 1024, 256, 64
NCORES = 8
RPC = B // NCORES  # 8192 rows per core
TILE_ROWS = 128
NTILES = RPC // TILE_ROWS  # 64
NCHUNK = Q // 128  # 8
NEG = -1.0e30

_NC_CACHE = {}


def build_nc():
    nc = bacc.Bacc(None, target_bir_lowering=False)
    f32 = mybir.dt.float32
    bf16 = mybir.dt.bfloat16

    x_ext = nc.declare_dram_parameter("x", [RPC, Q], f32, isOutput=False)
    kdt_ext = nc.declare_dram_parameter("kdt", [Q, MAXK], f32, isOutput=False)
    cbt_ext = nc.declare_dram_parameter("cbt", [Q, E], f32, isOutput=False)
    out_ext = nc.declare_dram_parameter("out", [RPC, E], f32, isOutput=True)

    with tile.TileContext(nc) as tc:
        with (
            tc.tile_pool(name="consts", bufs=1) as cpool,
            tc.tile_pool(name="work", bufs=6) as wpool,
            tc.tile_pool(name="ps_xt", bufs=2, space="PSUM") as ppool_xt,
            tc.tile_pool(name="ps_mm", bufs=2, space="PSUM") as ppool_mm,
        ):
            ident = cpool.tile([128, 128], f32)
            make_identity(nc, ident)

            # kdecider_w.T [1024, 64] -> chunks on partitions: [128, 8*64]
            kdt_sb = cpool.tile([128, NCHUNK * MAXK], f32)
            nc.sync.dma_start(
                kdt_sb.rearrange("p (c n) -> p c n", c=NCHUNK),
                kdt_ext.rearrange("(c p) n -> p c n", p=128),
            )
            # codebook_w.T [1024, 256] -> [128, 8*256], converted to bf16
            cbt_f32 = cpool.tile([128, NCHUNK * E], f32)
            nc.sync.dma_start(
                cbt_f32.rearrange("p (c n) -> p c n", c=NCHUNK),
                cbt_ext.rearrange("(c p) n -> p c n", p=128),
            )
            cbt_sb = cpool.tile([128, NCHUNK * E], bf16)
            nc.vector.tensor_copy(cbt_sb, cbt_f32)

            for i in range(NTILES):
                r0 = i * TILE_ROWS
                x_sb = wpool.tile([128, Q], f32, tag="x")
                nc.sync.dma_start(x_sb, x_ext[r0 : r0 + TILE_ROWS, :])

                # ---- transpose x -> xT (8x PE transpose + copies out) ----
                xT_ps = ppool_xt.tile([128, Q], f32, tag="xT_ps")
                for c in range(NCHUNK):
                    nc.tensor.transpose(
                        xT_ps[:, c * 128 : (c + 1) * 128],
                        x_sb[:, c * 128 : (c + 1) * 128],
                        ident,
                    )
                xT_sb = wpool.tile([128, Q], f32, tag="xT")
                for c in range(NCHUNK):
                    nc.scalar.copy(
                        xT_sb[:, c * 128 : (c + 1) * 128],
                        xT_ps[:, c * 128 : (c + 1) * 128],
                    )

                # ---- kd = x @ kdT (fp32, accumulate over 8 chunks) ----
                kd_ps = ppool_mm.tile([128, MAXK], f32, tag="kd_ps")
                for c in range(NCHUNK):
                    nc.tensor.matmul(
                        kd_ps,
                        xT_sb[:, c * 128 : (c + 1) * 128],
                        kdt_sb[:, c * MAXK : (c + 1) * MAXK],
                        start=(c == 0),
                        stop=(c == NCHUNK - 1),
                    )
                kd_sb = wpool.tile([128, MAXK], f32, tag="kd_sb")
                nc.scalar.copy(kd_sb, kd_ps)

                # one-hot of argmax(kd) (row max; ties ~impossible)
                kdm8 = wpool.tile([128, 8], f32, tag="kdm8")
                nc.vector.max(out=kdm8, in_=kd_sb)
                oh = wpool.tile([128, MAXK], f32, tag="oh")
                nc.vector.tensor_scalar(
                    oh, kd_sb, kdm8[:, 0:1], None, op0=mybir.AluOpType.is_ge
                )

                # ---- top-64 of each row: 8 rounds max8 + match_replace ----
                top64 = wpool.tile([128, MAXK], f32, tag="top64")
                wbuf = wpool.tile([128, Q], f32, tag="wbuf")
                cur = x_sb
                for g in range(8):
                    nc.vector.max(out=top64[:, g * 8 : (g + 1) * 8], in_=cur)
                    nc.vector.match_replace(
                        out=wbuf,
                        in_to_replace=top64[:, g * 8 : (g + 1) * 8],
                        in_values=cur,
                        imm_value=NEG,
                    )
                    cur = wbuf

                # ---- threshold t = sum(top64 * onehot)  [128, 1] ----
                prod = wpool.tile([128, MAXK], f32, tag="prod")
                nc.vector.tensor_mul(prod, top64, oh)
                tthr = wpool.tile([128, 1], f32, tag="tthr")
                nc.vector.reduce_sum(tthr, prod, axis=mybir.AxisListType.X)

                # ---- mask in natural layout: mask[r, q] = x[r, q] >= t[r]
                # (per-partition scalar threshold, single DVE op, bf16 out)
                mask_nat = wpool.tile([128, Q], bf16, tag="mask_nat")
                nc.vector.tensor_scalar(
                    mask_nat, x_sb, tthr[:, 0:1], None, op0=mybir.AluOpType.is_ge
                )

                # transpose mask via DMA xbar (2-byte dtype, HWDGE)
                maskT = wpool.tile([128, Q], bf16, tag="maskT")
                for c in range(NCHUNK):
                    eng = nc.sync if c % 2 == 0 else nc.scalar
                    eng.dma_start(
                        maskT[:, c * 128 : (c + 1) * 128],
                        mask_nat[:, c * 128 : (c + 1) * 128],
                        transpose=True,
                    )

                # ---- out = mask @ cbT (bf16 inputs, fp32 accumulate) ----
                out_ps = ppool_mm.tile([128, E], f32, tag="out_ps")
                for c in range(NCHUNK):
                    nc.tensor.matmul(
                        out_ps,
                        maskT[:, c * 128 : (c + 1) * 128],
                        cbt_sb[:, c * E : (c + 1) * E],
                        start=(c == 0),
                        stop=(c == NCHUNK - 1),
                    )
                out_sb = wpool.tile([128, E], f32, tag="out_sb")
                nc.scalar.copy(out_sb, out_ps)
                nc.sync.dma_start(out_ext[r0 : r0 + TILE_ROWS, :], out_sb)

    nc.finalize()
    return nc


def get_nc():
    if "nc" not in _NC_CACHE:
        _NC_CACHE["nc"] = build_nc()
    return _NC_CACHE["nc"]


def kernel(x, codebook_w, kdecider_w):
    x = np.ascontiguousarray(np.asarray(x, dtype=np.float32))
    kdt = np.ascontiguousarray(np.asarray(kdecider_w, dtype=np.float32).T)
    cbt = np.ascontiguousarray(np.asarray(codebook_w, dtype=np.float32).T)
    nc = get_nc()
    in_maps = [
        {"x": x[i * RPC : (i + 1) * RPC], "kdt": kdt, "cbt": cbt}
        for i in range(NCORES)
    ]
    res = run_bass_kernel_spmd(nc, in_maps, core_ids=list(range(NCORES)))
    return np.concatenate([res.results[i]["out"] for i in range(NCORES)], axis=0)
